# revision 9
# baseline (speedup 1.0000x reference)
"""Trainium2 Bass kernel for the canonical Lp-ECE KDE calibration loss.

Reference computation:
    probs = softmax(input, axis=1)[:, :, ::8, ::8]       -> f [N=8192, C=19]
    y     = argmax(target, axis=1)[:, ::8, ::8]          -> [N]
    alphas = f/0.02 + 1
    log_kern[i,j] = log(f[i]) . (alphas[j]-1) - log_beta[j]   (diag = -inf)
    kern = exp(log_kern);  ratio = (kern @ onehot(y)) / rowsum(kern)
    loss = mean_i sum_c (ratio - f)^2

The O(N^2) part (two GEMMs + 67M exps) runs on 8 NeuronCores, row-sharded:
core k owns rows i in [k*1024, (k+1)*1024).  The j (kernel-center) axis is
rotated per core by k*1024 so the self-interaction diagonal lands at
jlocal == ilocal in [0, 1024) -- one SPMD program masks it at fixed spots.

v2 design (vs the 94us baseline):
  * GEMM1 (PE, fp32r 4-quadrant, 3-wide tile_position concurrency) emits
    t[j,i] = S*(lognum - log_beta) + B  where S = 128/ln2, B = 16250.
    The Schraudolph constants are folded into stat on the host, so:
  * exp is split across TWO engines per unit:
      - ACT: kern = Exp(t*scale + bias) with scale=1/S, bias=-B/S (free).
      - DVE: bits = uint16(max(t, 0)) bit-viewed as bf16 -- Schraudolph
        exp2 approximation (~3% per-element, cancels in num/den ratio;
        measured end-to-end loss err ~5e-5 in simulation).
    A dummy exp at t=0 pulls the ~2.7us ACT table load out of the stream.
  * GEMM2 (PE, bf16, 4 col-groups) accumulates kern_y^T / den into
    psB [116, 512] per i-chunk.
  * NO device epilogue: psB is DMA'd straight to DRAM (fp32); the host
    combines the 4 col groups, divides, and reduces (O(N*C) numpy, same
    complexity as the host-side softmax/lgamma preprocessing).
"""

import numpy as np
import ml_dtypes
from scipy.special import gammaln

import concourse.bass as bass
import concourse.bacc as bacc
import concourse.tile as tile
from concourse import mybir
from concourse.bass_utils import run_bass_kernel_spmd

BF16 = mybir.dt.bfloat16
F32 = mybir.dt.float32
F32R = mybir.dt.float32r
U16 = mybir.dt.uint16
NPBF16 = ml_dtypes.bfloat16

N = 8192          # total pixels after downsampling: 2*64*64
C = 19            # classes
C1 = C + 1        # classes + ones column (row-sum)
NCORES = 8
R = N // NCORES   # rows per core = 1024
K1 = 20           # f32r contraction rows: 19 classes + 1 row for -log_beta
NT = N // 128     # j tiles = 64
BW = np.float32(0.02)
DF = 8
BIGNEG = -1.0e30

# Schraudolph folding: t = S*log_kern + B; bf16 bits = clamp(t, 0)
SCHRAUD_S = np.float64(128.0) / np.log(2.0)     # 184.6650062...
SCHRAUD_B = np.float64(16250.0)                 # 127*128 - c, c ~ 6

# unit -> exp engine assignment (True = DVE schraudolph, False = ACT exact)
# 22 units per i-chunk: [[0]], then 21 triples.
DVE_UNITS_C0 = {2, 4, 6, 8, 10, 12, 14, 16, 18, 20}
DVE_UNITS_C1 = {2, 4, 6, 8, 10, 12, 14, 16, 18}


def _build_nc():
    nc = bacc.Bacc(None, target_bir_lowering=False, debug=False)

    stat_d = nc.dram_tensor("stat", [128, N // 4], F32R, kind="ExternalInput")
    mov_d = nc.dram_tensor("mov", [128, R], F32R, kind="ExternalInput")
    y1_d = nc.dram_tensor("y1", [128, NT, C1], BF16, kind="ExternalInput")
    out0_d = nc.dram_tensor("out0", [116, 512], F32, kind="ExternalOutput")
    out1_d = nc.dram_tensor("out1", [116, 512], F32, kind="ExternalOutput")

    diag_np = (np.eye(128) * BIGNEG).astype(np.float32)
    diag_d = nc.inline_tensor(diag_np, name="diagmask")

    from contextlib import ExitStack

    with tile.TileContext(nc) as tc, ExitStack() as ctx:
        consts = ctx.enter_context(tc.tile_pool(name="consts", bufs=1))
        kpool = ctx.enter_context(tc.tile_pool(name="kern", bufs=6))
        pln = ctx.enter_context(
            tc.tile_pool(name="pln", bufs=2, space=bass.MemorySpace.PSUM)
        )
        pacc = ctx.enter_context(
            tc.tile_pool(name="pacc", bufs=2, space=bass.MemorySpace.PSUM)
        )

        # ---- warmup: force the ACT exp table load at t~0 ----
        warm_in = consts.tile([1, 16], F32)
        nc.vector.memset(warm_in[:], 0.0)
        warm_out = consts.tile([1, 16], BF16)
        nc.scalar.activation(
            warm_out[:], warm_in[:], mybir.ActivationFunctionType.Exp
        )
        # per-partition bias AP for the exp rescale (scale is an immediate)
        bexp_sb = consts.tile([128, 1], F32)
        nc.vector.memset(bexp_sb[:], float(-SCHRAUD_B / SCHRAUD_S))

        # ---- input DMAs, ordered by first use ----
        # sync (SP) queue: what the first units need.
        mov_sb = consts.tile([128, R], F32R)
        nc.sync.dma_start(out=mov_sb[:, 0:512], in_=mov_d[:, 0:512])
        stat_sb = consts.tile([128, N // 4], F32R)
        nc.sync.dma_start(out=stat_sb[:, 0:128], in_=stat_d[:, 0:128])
        diag_sb = consts.tile([128, 128], F32)
        nc.sync.dma_start(out=diag_sb[:], in_=diag_d[:])
        nc.sync.dma_start(out=stat_sb[:, 128:512], in_=stat_d[:, 128:512])
        nc.sync.dma_start(out=stat_sb[:, 512:2048], in_=stat_d[:, 512:2048])
        # gpsimd (Pool) queue: y1 head first (needed by first GEMM2), rest after.
        y1_sb = consts.tile([128, NT, C1], BF16)
        nc.gpsimd.dma_start(out=y1_sb[:, 0:8, :], in_=y1_d[:, 0:8, :])
        nc.gpsimd.dma_start(out=y1_sb[:, 8:NT, :], in_=y1_d[:, 8:NT, :])
        nc.gpsimd.dma_start(out=mov_sb[:, 512:R], in_=mov_d[:, 512:R])

        UNITS = [[0]] + [list(range(s, s + 3)) for s in range(1, NT, 3)]

        def emit_units(ic, psB, units, u_base):
            isl = slice(ic * 512, (ic + 1) * 512)
            dve_set = DVE_UNITS_C0 if ic == 0 else DVE_UNITS_C1
            for du, ts_list in enumerate(units):
                ui = u_base + du
                w = len(ts_list)
                unit = pln.tile([128, 512 * w], F32, tag="unit")
                for pos, t in enumerate(ts_list):
                    g = t % 4
                    tloc = t // 4
                    nc.tensor.matmul(
                        unit[:, pos * 512 : (pos + 1) * 512],
                        lhsT=stat_sb[
                            32 * g : 32 * g + K1, tloc * 128 : (tloc + 1) * 128
                        ],
                        rhs=mov_sb[32 * g : 32 * g + K1, isl],
                        start=True,
                        stop=True,
                        tile_position=(32 * g, 0),
                        skip_group_check=True,
                    )
                for pos, t in enumerate(ts_list):
                    if 4 * ic <= t < 4 * ic + 4:
                        off = pos * 512 + t * 128 - ic * 512
                        nc.vector.tensor_add(
                            unit[:, off : off + 128],
                            unit[:, off : off + 128],
                            diag_sb[:],
                        )
                ksb = kpool.tile([128, 512 * w], BF16, tag="ksb")
                if ui in dve_set:
                    nc.vector.tensor_scalar_max(
                        ksb[:].bitcast(U16), unit[:], 0.0
                    )
                else:
                    nc.scalar.activation(
                        ksb[:],
                        unit[:],
                        mybir.ActivationFunctionType.Exp,
                        bias=bexp_sb[:],
                        scale=float(1.0 / SCHRAUD_S),
                    )
                for pos, t in enumerate(ts_list):
                    g = t % 4
                    nc.tensor.matmul(
                        psB[32 * g : 32 * g + C1, :],
                        lhsT=y1_sb[:, t, :],
                        rhs=ksb[:, pos * 512 : (pos + 1) * 512],
                        start=(t < 4),
                        stop=(t >= NT - 4),
                        skip_group_check=True,
                        tile_position=(0, 32 * g),
                    )

        psB0 = pacc.tile([128, 512], F32, tag="psB")
        emit_units(0, psB0, UNITS, 0)
        psB1 = pacc.tile([128, 512], F32, tag="psB")
        emit_units(1, psB1, UNITS[:7], 0)
        # chunk-0 result out: PSUM -> SBUF copy, then 4 column-slice DMAs on
        # the sync queue, all overlapped with chunk-1 compute.
        ko0 = consts.tile([116, 512], F32)
        nc.scalar.copy(ko0[:, 0:256], psB0[0:116, 0:256])
        nc.vector.tensor_copy(ko0[:, 256:512], psB0[0:116, 256:512])
        for q in range(4):
            cs = slice(q * 128, (q + 1) * 128)
            nc.sync.dma_start(out=out0_d[:, cs], in_=ko0[:, cs])
        emit_units(1, psB1, UNITS[7:], 7)
        # chunk-1 (tail): split the copy across ACT and DVE to shorten the
        # critical path, then 8 DMA slices spread across 4 engine queues.
        ko1 = consts.tile([116, 512], F32)
        nc.scalar.copy(ko1[:, 0:256], psB1[0:116, 0:256])
        nc.vector.tensor_copy(ko1[:, 256:512], psB1[0:116, 256:512])
        engs = [nc.sync, nc.gpsimd, nc.scalar]
        for q in range(6):
            cs = slice(q * 86, min((q + 1) * 86, 512))
            engs[q % 3].dma_start(out=out1_d[:, cs], in_=ko1[:, cs])

    nc.compile()
    return nc


_NC_CACHE = None


def _get_nc():
    global _NC_CACHE
    if _NC_CACHE is None:
        _NC_CACHE = _build_nc()
    return _NC_CACHE


def prepare_in_maps(input, target):
    """Host-side preprocessing: softmax/argmax on the strided subsample,
    log-Beta normalizers, Schraudolph scale folding, and the per-core
    j-rotated layouts.  Returns (in_maps, f) -- f is reused by the host
    epilogue."""
    x = np.asarray(input)[:, :, ::DF, ::DF].astype(np.float32)
    t = np.asarray(target)[:, :, ::DF, ::DF]

    m = x.max(axis=1, keepdims=True)
    e = np.exp(x - m)
    probs = e / e.sum(axis=1, keepdims=True)
    f = probs.transpose(0, 2, 3, 1).reshape(-1, C).astype(np.float32)
    y = t.argmax(axis=1).reshape(-1)

    alphas = f / BW + np.float32(1.0)
    b = (alphas - np.float32(1.0)).astype(np.float64) * SCHRAUD_S
    logf = np.log(f)
    lb = (
        gammaln(alphas.astype(np.float64)).sum(axis=1)
        - gammaln(alphas.sum(axis=1, dtype=np.float64))
    )
    nlb = (-lb) * SCHRAUD_S + SCHRAUD_B

    b = b.astype(np.float32)
    nlb = nlb.astype(np.float32)

    in_maps = []
    for k in range(NCORES):
        perm = (np.arange(N) + k * R) % N
        b_rot = b[perm]
        nlb_rot = nlb[perm]
        # j-tile t lives in row-group t%4 (partitions 32g..32g+19), packed
        # at column block t//4
        stat = np.zeros((128, N // 4), dtype=np.float32)
        for g in range(4):
            sel = np.arange(NT // 4) * 4 + g          # tiles in this group
            cols = (sel[:, None] * 128 + np.arange(128)[None, :]).ravel()
            stat[32 * g : 32 * g + 19] = b_rot[cols].T
            stat[32 * g + 19] = nlb_rot[cols]

        rows = slice(k * R, (k + 1) * R)
        mov = np.zeros((128, R), dtype=np.float32)
        for g in range(4):
            mov[32 * g : 32 * g + 19] = logf[rows].T
            mov[32 * g + 19] = 1.0

        yp = y[perm].reshape(NT, 128)  # [t, p]
        y1 = np.zeros((128, NT, C1), dtype=NPBF16)
        onehot = (yp[:, :, None] == np.arange(C)[None, None, :]).astype(NPBF16)
        y1[:, :, 0:C] = onehot.transpose(1, 0, 2)
        y1[:, :, C] = NPBF16(1.0)

        in_maps.append({"stat": stat, "mov": mov, "y1": y1})
    return in_maps, f


def host_epilogue(results, f):
    """Combine per-core psB outputs into the scalar loss.
    psB row 32g+c = sum over j-tiles t = g (mod 4) of kern_y (c<19) / den
    (c=19) partials; columns of out{0,1} are i-cols of the core's chunk."""
    total = np.float64(0.0)
    for k, r in enumerate(results):
        ky = np.zeros((C1, R), dtype=np.float64)
        for ic, key in ((0, "out0"), (1, "out1")):
            blk = r[key].astype(np.float64)  # [116, 512]
            for g in range(4):
                ky[:, ic * 512 : (ic + 1) * 512] += blk[32 * g : 32 * g + C1]
        den = ky[C]
        ratio = ky[0:C] / den[None, :]
        fc = f[k * R : (k + 1) * R].T.astype(np.float64)  # [C, R]
        total += ((ratio - fc) ** 2).sum()
    return np.array(np.float32(total / N), dtype=np.float32)


def run_device(in_maps, trace=False, trace_cores=None):
    nc = _get_nc()
    return run_bass_kernel_spmd(
        nc,
        in_maps,
        core_ids=list(range(NCORES)),
        trace=trace,
        trace_cores=trace_cores,
    )


def kernel(input, target):
    in_maps, f = prepare_in_maps(input, target)
    res = run_device(in_maps)
    return host_epilogue(res.results, f)


# revision 12
# speedup vs baseline: 1.1372x; 1.1372x over previous
"""Trainium2 Bass kernel for the canonical Lp-ECE KDE calibration loss.

Reference computation:
    probs = softmax(input, axis=1)[:, :, ::8, ::8]       -> f [N=8192, C=19]
    y     = argmax(target, axis=1)[:, ::8, ::8]          -> [N]
    alphas = f/0.02 + 1
    log_kern[i,j] = log(f[i]) . (alphas[j]-1) - log_beta[j]   (diag = -inf)
    kern = exp(log_kern);  ratio = (kern @ onehot(y)) / rowsum(kern)
    loss = mean_i sum_c (ratio - f)^2

The O(N^2) part (two GEMMs + 67M exps) runs on 8 NeuronCores, row-sharded:
core k owns rows i in [k*1024, (k+1)*1024).  The j (kernel-center) axis is
rotated per core by k*1024 so the self-interaction diagonal lands at
jlocal == ilocal in [0, 1024) -- one SPMD program masks it at fixed spots.

v2 design (vs the 94us baseline):
  * GEMM1 (PE, fp32r 4-quadrant, 3-wide tile_position concurrency) emits
    t[j,i] = S*(lognum - log_beta) + B  where S = 128/ln2, B = 16250.
    The Schraudolph constants are folded into stat on the host, so:
  * exp is split across TWO engines per unit:
      - ACT: kern = Exp(t*scale + bias) with scale=1/S, bias=-B/S (free).
      - DVE: bits = uint16(max(t, 0)) bit-viewed as bf16 -- Schraudolph
        exp2 approximation (~3% per-element, cancels in num/den ratio;
        measured end-to-end loss err ~5e-5 in simulation).
    A dummy exp at t=0 pulls the ~2.7us ACT table load out of the stream.
  * GEMM2 (PE, bf16, 4 col-groups) accumulates kern_y^T / den into
    psB [116, 512] per i-chunk.
  * NO device epilogue: psB is DMA'd straight to DRAM (fp32); the host
    combines the 4 col groups, divides, and reduces (O(N*C) numpy, same
    complexity as the host-side softmax/lgamma preprocessing).
"""

import numpy as np
import ml_dtypes
from scipy.special import gammaln

import concourse.bass as bass
import concourse.bacc as bacc
import concourse.tile as tile
from concourse import mybir
from concourse.bass_utils import run_bass_kernel_spmd

BF16 = mybir.dt.bfloat16
F32 = mybir.dt.float32
F32R = mybir.dt.float32r
U16 = mybir.dt.uint16
NPBF16 = ml_dtypes.bfloat16

N = 8192          # total pixels after downsampling: 2*64*64
C = 19            # classes
C1 = C + 1        # classes + ones column (row-sum)
NCORES = 8
R = N // NCORES   # rows per core = 1024
K1 = 20           # f32r contraction rows: 19 classes + 1 row for -log_beta
NT = N // 128     # j tiles = 64
BW = np.float32(0.02)
DF = 8
BIGNEG = -1.0e30

# Schraudolph folding: t = S*log_kern + B; bf16 bits = clamp(t, 0)
SCHRAUD_S = np.float64(128.0) / np.log(2.0)     # 184.6650062...
SCHRAUD_B = np.float64(16250.0)                 # 127*128 - c, c ~ 6

# unit -> exp engine assignment (True = DVE schraudolph, False = ACT exact)
# 22 units per i-chunk: [[0]], then 21 triples.
DVE_UNITS_C0 = {2, 4, 6, 8, 10, 12, 14, 16, 18, 20}
DVE_UNITS_C1 = {2, 4, 6, 8, 10, 12, 14, 16, 18}


def _build_nc():
    nc = bacc.Bacc(None, target_bir_lowering=False, debug=False)

    stat_d = nc.dram_tensor("stat", [128, N // 4], F32R, kind="ExternalInput")
    mov_d = nc.dram_tensor("mov", [128, R], F32R, kind="ExternalInput")
    y1_d = nc.dram_tensor("y1", [128, NT, C1], BF16, kind="ExternalInput")
    out0_d = nc.dram_tensor("out0", [116, 512], F32, kind="ExternalOutput")
    out1_d = nc.dram_tensor("out1", [116, 512], F32, kind="ExternalOutput")

    diag_np = (np.eye(128) * BIGNEG).astype(np.float32)
    diag_d = nc.inline_tensor(diag_np, name="diagmask")

    from contextlib import ExitStack

    with tile.TileContext(nc) as tc, ExitStack() as ctx:
        consts = ctx.enter_context(tc.tile_pool(name="consts", bufs=1))
        kpool = ctx.enter_context(tc.tile_pool(name="kern", bufs=6))
        pln = ctx.enter_context(
            tc.tile_pool(name="pln", bufs=2, space=bass.MemorySpace.PSUM)
        )
        pacc = ctx.enter_context(
            tc.tile_pool(name="pacc", bufs=2, space=bass.MemorySpace.PSUM)
        )

        # ---- warmup: force the ACT exp table load at t~0 ----
        warm_in = consts.tile([1, 16], F32)
        nc.vector.memset(warm_in[:], 0.0)
        warm_out = consts.tile([1, 16], BF16)
        nc.scalar.activation(
            warm_out[:], warm_in[:], mybir.ActivationFunctionType.Exp
        )
        # per-partition bias AP for the exp rescale (scale is an immediate)
        bexp_sb = consts.tile([128, 1], F32)
        nc.vector.memset(bexp_sb[:], float(-SCHRAUD_B / SCHRAUD_S))

        # ---- input DMAs ----
        # One SBUF tile per DMA so consumers wait only on the chunk they
        # read (dependency tracking is per-tile): the first GEMM1 needs just
        # statA [128,128] + movA [128,512].
        movA = consts.tile([128, 512], F32R)
        nc.sync.dma_start(out=movA[:], in_=mov_d[:, 0:512])
        statA = consts.tile([128, 128], F32R)
        nc.sync.dma_start(out=statA[:], in_=stat_d[:, 0:128])
        diag_sb = consts.tile([128, 128], F32)
        nc.scalar.dma_start(out=diag_sb[:], in_=diag_d[:])
        statB = consts.tile([128, 384], F32R)
        nc.scalar.dma_start(out=statB[:], in_=stat_d[:, 128:512])
        y1A = consts.tile([128, 8, C1], BF16)
        nc.gpsimd.dma_start(out=y1A[:], in_=y1_d[:, 0:8, :])
        statC = consts.tile([128, 1536], F32R)
        nc.sync.dma_start(out=statC[:, 0:768], in_=stat_d[:, 512:1280])
        nc.gpsimd.dma_start(out=statC[:, 768:1536], in_=stat_d[:, 1280:2048])
        y1B = consts.tile([128, NT - 8, C1], BF16)
        nc.gpsimd.dma_start(out=y1B[:], in_=y1_d[:, 8:NT, :])
        movB = consts.tile([128, 512], F32R)
        nc.gpsimd.dma_start(out=movB[:], in_=mov_d[:, 512:R])

        def stat_ap(g, tloc):
            rows = slice(32 * g, 32 * g + K1)
            if tloc < 1:
                return statA[rows, :]
            if tloc < 4:
                return statB[rows, (tloc - 1) * 128 : tloc * 128]
            return statC[rows, (tloc - 4) * 128 : (tloc - 3) * 128]

        def y1_ap(t):
            return y1A[:, t, :] if t < 8 else y1B[:, t - 8, :]

        UNITS = [[0]] + [list(range(s, s + 3)) for s in range(1, NT, 3)]

        def emit_gemm2(carry):
            ksb, ts_list, psB = carry
            for pos, t in enumerate(ts_list):
                g = t % 4
                nc.tensor.matmul(
                    psB[32 * g : 32 * g + C1, :],
                    lhsT=y1_ap(t),
                    rhs=ksb[:, pos * 512 : (pos + 1) * 512],
                    start=(t < 4),
                    stop=(t >= NT - 4),
                    skip_group_check=True,
                    tile_position=(0, 32 * g),
                )

        def emit_units(ic, psB, units, u_base, carry):
            # GEMM2 for unit n is emitted during unit n+1 (after its exp),
            # keeping <=4 dependency-parked matmuls in the PE's reorder
            # window so later GEMM1s keep dispatching.
            mov = movA if ic == 0 else movB
            dve_set = DVE_UNITS_C0 if ic == 0 else DVE_UNITS_C1
            for du, ts_list in enumerate(units):
                ui = u_base + du
                w = len(ts_list)
                unit = pln.tile([128, 512 * w], F32, tag="unit")
                for pos, t in enumerate(ts_list):
                    g = t % 4
                    nc.tensor.matmul(
                        unit[:, pos * 512 : (pos + 1) * 512],
                        lhsT=stat_ap(g, t // 4),
                        rhs=mov[32 * g : 32 * g + K1, :],
                        start=True,
                        stop=True,
                        tile_position=(32 * g, 0),
                        skip_group_check=True,
                    )
                for pos, t in enumerate(ts_list):
                    if 4 * ic <= t < 4 * ic + 4:
                        off = pos * 512 + t * 128 - ic * 512
                        nc.vector.tensor_add(
                            unit[:, off : off + 128],
                            unit[:, off : off + 128],
                            diag_sb[:],
                        )
                ksb = kpool.tile([128, 512 * w], BF16, tag="ksb")
                if ui in dve_set:
                    nc.vector.tensor_scalar_max(
                        ksb[:].bitcast(U16), unit[:], 0.0
                    )
                else:
                    nc.scalar.activation(
                        ksb[:],
                        unit[:],
                        mybir.ActivationFunctionType.Exp,
                        bias=bexp_sb[:],
                        scale=float(1.0 / SCHRAUD_S),
                    )
                if carry is not None:
                    emit_gemm2(carry)
                carry = (ksb, ts_list, psB)
            return carry

        psB0 = pacc.tile([128, 512], F32, tag="psB")
        carry = emit_units(0, psB0, UNITS, 0, None)
        psB1 = pacc.tile([128, 512], F32, tag="psB")
        carry = emit_units(1, psB1, UNITS[:7], 0, carry)
        # chunk-0 result out: PSUM -> SBUF copy, then 4 column-slice DMAs on
        # the sync queue, all overlapped with chunk-1 compute.
        ko0 = consts.tile([116, 512], F32)
        nc.scalar.copy(ko0[:, 0:256], psB0[0:116, 0:256])
        nc.vector.tensor_copy(ko0[:, 256:512], psB0[0:116, 256:512])
        for q in range(4):
            cs = slice(q * 128, (q + 1) * 128)
            nc.sync.dma_start(out=out0_d[:, cs], in_=ko0[:, cs])
        carry = emit_units(1, psB1, UNITS[7:], 7, carry)
        emit_gemm2(carry)
        # chunk-1 (tail): split the copy across ACT and DVE to shorten the
        # critical path, then 8 DMA slices spread across 4 engine queues.
        ko1 = consts.tile([116, 512], F32)
        nc.scalar.copy(ko1[:, 0:256], psB1[0:116, 0:256])
        nc.vector.tensor_copy(ko1[:, 256:512], psB1[0:116, 256:512])
        engs = [nc.sync, nc.gpsimd, nc.scalar]
        for q in range(6):
            cs = slice(q * 86, min((q + 1) * 86, 512))
            engs[q % 3].dma_start(out=out1_d[:, cs], in_=ko1[:, cs])

    nc.compile()
    return nc


_NC_CACHE = None


def _get_nc():
    global _NC_CACHE
    if _NC_CACHE is None:
        _NC_CACHE = _build_nc()
    return _NC_CACHE


def prepare_in_maps(input, target):
    """Host-side preprocessing: softmax/argmax on the strided subsample,
    log-Beta normalizers, Schraudolph scale folding, and the per-core
    j-rotated layouts.  Returns (in_maps, f) -- f is reused by the host
    epilogue."""
    x = np.asarray(input)[:, :, ::DF, ::DF].astype(np.float32)
    t = np.asarray(target)[:, :, ::DF, ::DF]

    m = x.max(axis=1, keepdims=True)
    e = np.exp(x - m)
    probs = e / e.sum(axis=1, keepdims=True)
    f = probs.transpose(0, 2, 3, 1).reshape(-1, C).astype(np.float32)
    y = t.argmax(axis=1).reshape(-1)

    alphas = f / BW + np.float32(1.0)
    b = (alphas - np.float32(1.0)).astype(np.float64) * SCHRAUD_S
    logf = np.log(f)
    lb = (
        gammaln(alphas.astype(np.float64)).sum(axis=1)
        - gammaln(alphas.sum(axis=1, dtype=np.float64))
    )
    nlb = (-lb) * SCHRAUD_S + SCHRAUD_B

    b = b.astype(np.float32)
    nlb = nlb.astype(np.float32)

    in_maps = []
    for k in range(NCORES):
        perm = (np.arange(N) + k * R) % N
        b_rot = b[perm]
        nlb_rot = nlb[perm]
        # j-tile t lives in row-group t%4 (partitions 32g..32g+19), packed
        # at column block t//4
        stat = np.zeros((128, N // 4), dtype=np.float32)
        for g in range(4):
            sel = np.arange(NT // 4) * 4 + g          # tiles in this group
            cols = (sel[:, None] * 128 + np.arange(128)[None, :]).ravel()
            stat[32 * g : 32 * g + 19] = b_rot[cols].T
            stat[32 * g + 19] = nlb_rot[cols]

        rows = slice(k * R, (k + 1) * R)
        mov = np.zeros((128, R), dtype=np.float32)
        for g in range(4):
            mov[32 * g : 32 * g + 19] = logf[rows].T
            mov[32 * g + 19] = 1.0

        yp = y[perm].reshape(NT, 128)  # [t, p]
        y1 = np.zeros((128, NT, C1), dtype=NPBF16)
        onehot = (yp[:, :, None] == np.arange(C)[None, None, :]).astype(NPBF16)
        y1[:, :, 0:C] = onehot.transpose(1, 0, 2)
        y1[:, :, C] = NPBF16(1.0)

        in_maps.append({"stat": stat, "mov": mov, "y1": y1})
    return in_maps, f


def host_epilogue(results, f):
    """Combine per-core psB outputs into the scalar loss.
    psB row 32g+c = sum over j-tiles t = g (mod 4) of kern_y (c<19) / den
    (c=19) partials; columns of out{0,1} are i-cols of the core's chunk."""
    total = np.float64(0.0)
    for k, r in enumerate(results):
        ky = np.zeros((C1, R), dtype=np.float64)
        for ic, key in ((0, "out0"), (1, "out1")):
            blk = r[key].astype(np.float64)  # [116, 512]
            for g in range(4):
                ky[:, ic * 512 : (ic + 1) * 512] += blk[32 * g : 32 * g + C1]
        den = ky[C]
        ratio = ky[0:C] / den[None, :]
        fc = f[k * R : (k + 1) * R].T.astype(np.float64)  # [C, R]
        total += ((ratio - fc) ** 2).sum()
    return np.array(np.float32(total / N), dtype=np.float32)


def run_device(in_maps, trace=False, trace_cores=None):
    nc = _get_nc()
    return run_bass_kernel_spmd(
        nc,
        in_maps,
        core_ids=list(range(NCORES)),
        trace=trace,
        trace_cores=trace_cores,
    )


def kernel(input, target):
    in_maps, f = prepare_in_maps(input, target)
    res = run_device(in_maps)
    return host_epilogue(res.results, f)
